# revision 1
# baseline (speedup 1.0000x reference)
"""Causal self-attention Bass/Tile kernel for 8 TRN2 NeuronCores.

Sharding: core c handles batch b = c//2 and heads h in [8*(c%2), 8*(c%2)+8).
Each core computes a partial projection output (its 512 channels' worth of the
contraction); the host sums the two partials per batch.

Per-core pipeline (per 512-wide t-chunk):
  xT  [c,t] bf16   <- hardware DMA-transpose of host-cast bf16 x
  wT  [c,j] bf16   <- DMA-transpose of host-cast bf16 wqkv (one-time)
  q,k [j,t] f32r   <- bf16 matmul (lhsT=wT, rhs=xT), psum->sbuf cast
  v   [t,j] bf16   <- bf16 matmul (lhsT=xT, rhs=wT_v), with ones column for l
  S.T [tk,tq] psum <- f32r matmul, 2 heads row-tiled; causal mask pre-added
                      into psum via identity@mask matmul on diagonal tiles
  P = exp(S/8)     <- single ACT call per tk-tile (both heads), bf16 out
  y.T|l  psum      <- bf16 AV matmul accumulation; scores emitted one tile
                      ahead of AV so ACT never stalls the PE queue
  y.T/l            <- l broadcast via K=2 select-matmul, DVE divide
  out [t,c] f32    <- f32r proj matmul from y.T tiles, DMA to DRAM
"""

import sys

if "/opt/trn_rl_repo" not in sys.path:
    sys.path.insert(0, "/opt/trn_rl_repo")

import ml_dtypes
import numpy as np

import concourse.bass as bass
import concourse.mybir as mybir
import concourse.tile as tile
from concourse import bacc, bass_utils
from concourse.masks import make_identity

F32 = mybir.dt.float32
F32R = mybir.dt.float32r
BF16 = mybir.dt.bfloat16

B, T, C = 4, 2048, 1024
H = 16
D = 64
JL = 512          # local channels per q/k/v slice (8 heads * 64)
P = 128
NCHUNK = T // 512
NPAIR = 4
NEG = -1e30


def build_nc():
    nc = bacc.Bacc("TRN2", target_bir_lowering=False, debug=False)
    xb = nc.dram_tensor("xb", [T, C], BF16, kind="ExternalInput").ap()
    wqkvb = nc.dram_tensor("wqkvb", [3 * JL, C], BF16, kind="ExternalInput").ap()
    wproj = nc.dram_tensor("wproj", [C, JL], F32, kind="ExternalInput").ap()
    out = nc.dram_tensor("out", [T, C], F32, kind="ExternalOutput").ap()

    CT = C // P       # 8 c-tiles
    Exp = mybir.ActivationFunctionType.Exp
    Copy = mybir.ActivationFunctionType.Copy

    with tile.TileContext(nc) as tc:
        with (
            tc.tile_pool(name="singles", bufs=1) as singles,
            tc.tile_pool(name="stage", bufs=2) as stage_pool,
            tc.tile_pool(name="xT", bufs=2) as xT_pool,
            tc.tile_pool(name="qsb", bufs=2) as qsb_pool,
            tc.tile_pool(name="pt", bufs=4) as pt_pool,
            tc.tile_pool(name="yT", bufs=2) as yT_pool,
            tc.tile_pool(name="ob", bufs=3) as ob_pool,
            tc.tile_pool(name="small", bufs=3) as small_pool,
            tc.tile_pool(name="ps_sc", bufs=2, space="PSUM") as ps_sc,
            tc.tile_pool(name="ps_av", bufs=2, space="PSUM") as ps_av,
            tc.tile_pool(name="ps_mm", bufs=2, space="PSUM") as ps_mm,
        ):
            identity = singles.tile([P, P], F32)
            make_identity(nc, identity)
            identity_r = singles.tile([P, P], F32R)
            nc.vector.tensor_copy(identity_r, identity)

            # head-pair selector for the l broadcast: out rows 0:64 <- l0,
            # rows 64:128 <- l1  (out = sel2.T @ [l0;l1])
            selaf = singles.tile([1, P], F32)
            nc.vector.memset(selaf, 0.0)
            nc.vector.memset(selaf[0:1, 0:D], 1.0)
            selbf = singles.tile([1, P], F32)
            nc.vector.memset(selbf, 0.0)
            nc.vector.memset(selbf[0:1, D:2 * D], 1.0)
            sel_a = singles.tile([1, P], F32R)
            nc.vector.tensor_copy(sel_a, selaf)
            sel_b = singles.tile([1, P], F32R)
            nc.vector.tensor_copy(sel_b, selbf)

            # additive causal masks, one per diagonal offset o=128*k:
            # mask[k][p, f] = 0 if f >= p + 128k else -1e30   (f in 0..511)
            maskf = singles.tile([P, 4, 512], F32)
            for k in range(4):
                nc.gpsimd.memset(maskf[:, k, :], 0.0)
                nc.gpsimd.affine_select(
                    out=maskf[:, k, :],
                    in_=maskf[:, k, :],
                    pattern=[[1, 512]],
                    compare_op=mybir.AluOpType.is_ge,
                    fill=NEG,
                    base=-(128 * k),
                    channel_multiplier=-1,
                )
            maskr = singles.tile([P, 4, 512], F32R)
            nc.vector.tensor_copy(maskr, maskf)

            # persistent tensors
            wT = singles.tile([P, 3, CT, JL], BF16)   # [c, qkv, cc, j] 24KB/part
            wprojT = singles.tile([P, 4, C], F32R)    # [j, g, c_out]  16KB/part
            k_sb = singles.tile([P, 4, T], F32R)      # [d2, hp, tk]   32KB/part
            v_sb = singles.tile([P, T // P, 8, D + 1], BF16)  # [t, tkt, h, d|1]
            nc.vector.memset(v_sb[:, :, :, D], 1.0)

            # ---- stage 0: weight loads (DMA transpose for bf16 wqkv) ----
            for g3 in range(3):
                for cc in range(CT):
                    nc.sync.dma_start_transpose(
                        wT[:, g3, cc, :],
                        wqkvb[g3 * JL:(g3 + 1) * JL, cc * P:(cc + 1) * P],
                    )
            # wproj via PE transpose (fp32 path, one-time 32 tiles)
            for ct in range(8):
                wp = stage_pool.tile([P, JL], F32, tag="wp")
                nc.sync.dma_start(out=wp, in_=wproj[ct * P:(ct + 1) * P, :])
                for g in range(4):
                    ptile = ps_mm.tile([P, P], F32, tag="mm")
                    nc.tensor.transpose(ptile, wp[:, g * P:(g + 1) * P], identity)
                    nc.vector.tensor_copy(
                        out=wprojT[:, g, ct * P:(ct + 1) * P], in_=ptile
                    )

            pending_div = None

            def emit_div(pend):
                pav0, pav1, yT_t, hp_ = pend
                # l rows (psum row 64) -> sbuf f32r via ACT: its queue sits
                # right behind the pair's last exp so the wait is short
                l2a = small_pool.tile([1, 512], F32R, tag="l2a")
                l2b = small_pool.tile([1, 512], F32R, tag="l2b")
                nc.scalar.activation(l2a, pav0[D:D + 1, :], Copy)
                nc.scalar.activation(l2b, pav1[D:D + 1, :], Copy)
                pb = ps_mm.tile([P, 512], F32, tag="mm")
                nc.tensor.matmul(pb, lhsT=sel_a, rhs=l2a,
                                 start=True, stop=False)
                nc.tensor.matmul(pb, lhsT=sel_b, rhs=l2b,
                                 start=False, stop=True)
                pbs = small_pool.tile([P, 512], F32, tag="pbs")
                nc.vector.reciprocal_approx_fast(out=pbs, in_=pb)
                nc.vector.tensor_mul(
                    yT_t[0:D, hp_, :], pav0[0:D, :], pbs[0:D, :]
                )
                nc.vector.tensor_mul(
                    yT_t[D:P, hp_, :], pav1[0:D, :], pbs[D:P, :]
                )

            for q in range(NCHUNK):
                t0 = q * 512
                # ---- QKV for t-chunk q ----
                xT = xT_pool.tile([P, CT, 512], BF16)
                for cc in range(CT):
                    nc.sync.dma_start_transpose(
                        xT[:, cc, :],
                        xb[t0:t0 + 512, cc * P:(cc + 1) * P],
                    )
                # v in [t, j] (+ ones col maintained by init memset)
                for tt in range(4):
                    pv = ps_mm.tile([P, JL], F32, tag="mm")
                    for cc in range(CT):
                        nc.tensor.matmul(
                            pv,
                            lhsT=xT[:, cc, tt * P:(tt + 1) * P],
                            rhs=wT[:, 2, cc, :],
                            start=(cc == 0),
                            stop=(cc == CT - 1),
                        )
                    for h in range(8):
                        nc.vector.tensor_copy(
                            out=v_sb[:, q * 4 + tt, h, 0:D],
                            in_=pv[:, h * D:(h + 1) * D],
                        )
                # q, k in [j, t]
                q_sb = qsb_pool.tile([P, 4, 512], F32R)
                for jt in range(8):
                    g3, j4 = (0, jt) if jt < 4 else (1, jt - 4)
                    pq = ps_mm.tile([P, 512], F32, tag="mm")
                    for cc in range(CT):
                        nc.tensor.matmul(
                            pq,
                            lhsT=wT[:, g3, cc, j4 * P:(j4 + 1) * P],
                            rhs=xT[:, cc, :],
                            start=(cc == 0),
                            stop=(cc == CT - 1),
                        )
                    if jt < 4:
                        nc.vector.tensor_copy(out=q_sb[:, jt, :], in_=pq)
                    else:
                        nc.vector.tensor_copy(
                            out=k_sb[:, jt - 4, t0:t0 + 512], in_=pq
                        )

                # ---- attention for tq-chunk q ----
                yT = yT_pool.tile([P, 4, 512], F32R)
                ntk = 4 * q + 4

                for hp in range(NPAIR):
                    pav0 = ps_av.tile([D + 1, 512], F32, tag="av")
                    pav1 = ps_av.tile([D + 1, 512], F32, tag="av")
                    pav = [pav0, pav1]

                    def emit_scores(j, hp=hp, q_sb=q_sb):
                        # diagonal tiles only need columns o:512
                        diag = j >= 4 * q
                        o = j * P - t0 if diag else 0
                        ps = ps_sc.tile([P, 2, 512], F32, tag="sc")
                        for h2 in range(2):
                            if diag:  # pre-add causal mask into psum
                                nc.tensor.matmul(
                                    ps[:, h2, o:512],
                                    lhsT=identity_r,
                                    rhs=maskr[:, o // P, o:512],
                                    start=True,
                                    stop=False,
                                )
                            nc.tensor.matmul(
                                ps[:, h2, o:512],
                                lhsT=k_sb[
                                    h2 * D:(h2 + 1) * D, hp, j * P:(j + 1) * P
                                ],
                                rhs=q_sb[h2 * D:(h2 + 1) * D, hp, o:512],
                                start=not diag,
                                stop=True,
                            )
                        return ps, o

                    sc_q = [emit_scores(0)]
                    for j in range(ntk):
                        if j + 1 < ntk:
                            sc_q.append(emit_scores(j + 1))
                        ps, o = sc_q[j]
                        pt = pt_pool.tile([P, 2, 512], BF16, tag="pt")
                        nc.scalar.activation(
                            pt[:, :, o:512], ps[:, :, o:512], Exp, scale=0.125
                        )
                        for h2 in range(2):
                            nc.tensor.matmul(
                                pav[h2][:, o:512],
                                lhsT=v_sb[:, j, hp * 2 + h2, :],
                                rhs=pt[:, h2, o:512],
                                start=(j == 0),
                                stop=(j == ntk - 1),
                            )
                    if pending_div is not None:
                        emit_div(pending_div)
                    pending_div = (pav0, pav1, yT, hp)

                # ---- proj for t-chunk q ----
                if pending_div is not None:
                    emit_div(pending_div)
                    pending_div = None
                for tt in range(4):
                    for ct in range(2):
                        po = ps_mm.tile([P, 512], F32, tag="mm")
                        for g in range(4):
                            nc.tensor.matmul(
                                po,
                                lhsT=yT[:, g, tt * P:(tt + 1) * P],
                                rhs=wprojT[:, g, ct * 512:(ct + 1) * 512],
                                start=(g == 0),
                                stop=(g == 3),
                            )
                        ob = ob_pool.tile([P, 512], F32, tag="ob")
                        nc.vector.tensor_copy(ob, po)
                        nc.sync.dma_start(
                            out=out[
                                t0 + tt * P:t0 + (tt + 1) * P,
                                ct * 512:(ct + 1) * 512,
                            ],
                            in_=ob,
                        )

    nc.compile()
    return nc


_NC = None


def _get_nc():
    global _NC
    if _NC is None:
        _NC = build_nc()
    return _NC


def _shard_inputs(x, w_attn, w_proj):
    in_maps = []
    for c in range(8):
        b, s = c // 2, c % 2
        j0 = s * JL
        wqkv_c = np.concatenate(
            [
                w_attn[j0:j0 + JL],
                w_attn[C + j0:C + j0 + JL],
                w_attn[2 * C + j0:2 * C + j0 + JL],
            ],
            axis=0,
        )
        in_maps.append(
            {
                "xb": np.ascontiguousarray(x[b]).astype(ml_dtypes.bfloat16),
                "wqkvb": np.ascontiguousarray(wqkv_c).astype(ml_dtypes.bfloat16),
                "wproj": np.ascontiguousarray(w_proj[:, j0:j0 + JL]).astype(
                    np.float32
                ),
            }
        )
    return in_maps


def run(x, w_attn, w_proj, **run_kwargs):
    """Run on 8 cores; returns (out [B,T,C], BassKernelResults)."""
    nc = _get_nc()
    in_maps = _shard_inputs(np.asarray(x), np.asarray(w_attn), np.asarray(w_proj))
    res = bass_utils.run_bass_kernel_spmd(
        nc, in_maps, core_ids=list(range(8)), **run_kwargs
    )
    out = np.empty((B, T, C), dtype=np.float32)
    for b in range(B):
        out[b] = res.results[2 * b]["out"] + res.results[2 * b + 1]["out"]
    return out, res


def kernel(x, w_attn, w_proj):
    return run(x, w_attn, w_proj)[0]



# revision 2
# speedup vs baseline: 1.4080x; 1.4080x over previous
"""Causal self-attention Bass/Tile kernel for 8 TRN2 NeuronCores.

Sharding: core c handles batch b = c//2 and heads h in [8*(c%2), 8*(c%2)+8).
Each core computes a partial projection output (its 512 channels' worth of the
contraction); the host sums the two partials per batch.

v2 changes vs baseline:
  - x, wqkv, wproj are pre-transposed on the HOST; all device DMA is direct
    (the baseline's 56 serial DMA_TRANSPOSEs cost ~70us of startup stall).
  - QKV matmuls for chunk q+1 and proj matmuls for chunk q-1 are interleaved
    as PE filler inside chunk q's attention loop, so the PE never idles while
    the ACT engine computes exp (ACT needs ~1.15us/tile vs PE ~0.65us/tile).
  - causal-mask matmuls cover only the 128-wide diagonal band (emitted after
    the score matmuls, accumulating into the same psum group).
  - softmax-denominator row copies moved from ACT (critical engine) to DVE.
  - v-tile psum->sbuf copies merged into one strided DVE op per t-tile.

Per-core pipeline (per 512-wide t-chunk):
  xT  [c,t] bf16   <- direct DMA of host-transposed bf16 x
  q,k [j,t] f32r   <- bf16 matmul (lhsT=wT, rhs=xT), psum->sbuf cast
  v   [t,j] bf16   <- bf16 matmul (lhsT=xT, rhs=wT_v), with ones column for l
  S.T [tk,tq] psum <- f32r matmul, 2 heads row-tiled (auto tile_position);
                      causal band mask added into psum on diagonal tiles
  P = exp(S/8)     <- single ACT call per tk-tile (both heads), bf16 out
  y.T|l  psum      <- bf16 AV matmul accumulation; scores emitted one tile
                      ahead of AV; QKV/proj filler keeps PE busy during exp
  y.T/l            <- l broadcast via K=1 select-matmul, DVE divide
  out [t,c] f32    <- f32r proj matmul from y.T tiles, DMA to DRAM
"""

import sys

if "/opt/trn_rl_repo" not in sys.path:
    sys.path.insert(0, "/opt/trn_rl_repo")

import ml_dtypes
import numpy as np

import concourse.bass as bass
import concourse.mybir as mybir
import concourse.tile as tile
from concourse import bacc, bass_utils
from concourse.masks import make_identity

F32 = mybir.dt.float32
F32R = mybir.dt.float32r
BF16 = mybir.dt.bfloat16

B, T, C = 4, 2048, 1024
H = 16
D = 64
JL = 512          # local channels per q/k/v slice (8 heads * 64)
P = 128
NCHUNK = T // 512
NPAIR = 4
NEG = -1e30


def build_nc():
    nc = bacc.Bacc("TRN2", target_bir_lowering=False, debug=False)
    # host-transposed layouts
    xb = nc.dram_tensor("xb", [C, T], BF16, kind="ExternalInput").ap()
    wqkvb = nc.dram_tensor("wqkvb", [C, 3 * JL], BF16, kind="ExternalInput").ap()
    wprojb = nc.dram_tensor("wprojb", [JL, C], F32, kind="ExternalInput").ap()
    out = nc.dram_tensor("out", [T, C], F32, kind="ExternalOutput").ap()

    CT = C // P       # 8 c-tiles
    Exp = mybir.ActivationFunctionType.Exp

    with tile.TileContext(nc) as tc:
        with (
            tc.tile_pool(name="singles", bufs=1) as singles,
            tc.tile_pool(name="scratch", bufs=1) as scratch,
            tc.tile_pool(name="xT", bufs=2) as xT_pool,
            tc.tile_pool(name="qsb", bufs=2) as qsb_pool,
            tc.tile_pool(name="pt", bufs=4) as pt_pool,
            tc.tile_pool(name="yT", bufs=2) as yT_pool,
            tc.tile_pool(name="ob", bufs=3) as ob_pool,
            tc.tile_pool(name="small", bufs=2) as small_pool,
            tc.tile_pool(name="ps_sc", bufs=2, space="PSUM") as ps_sc,
            tc.tile_pool(name="ps_av", bufs=2, space="PSUM") as ps_av,
            tc.tile_pool(name="ps_mm", bufs=2, space="PSUM") as ps_mm,
        ):
            identity = singles.tile([P, P], F32)
            make_identity(nc, identity)
            identity_r = singles.tile([P, P], F32R)
            nc.vector.tensor_copy(identity_r, identity)

            # head-pair selector for the l broadcast: out rows 0:64 <- l0,
            # rows 64:128 <- l1  (out = sel2.T @ [l0;l1])
            selaf = singles.tile([1, P], F32)
            nc.vector.memset(selaf, 0.0)
            nc.vector.memset(selaf[0:1, 0:D], 1.0)
            selbf = singles.tile([1, P], F32)
            nc.vector.memset(selbf, 0.0)
            nc.vector.memset(selbf[0:1, D:2 * D], 1.0)
            sel_a = singles.tile([1, P], F32R)
            nc.vector.tensor_copy(sel_a, selaf)
            sel_b = singles.tile([1, P], F32R)
            nc.vector.tensor_copy(sel_b, selbf)

            # additive causal masks, one per diagonal offset o=128*k:
            # mask[k][p, f] = 0 if f >= p + 128k else -1e30   (f in 0..511)
            maskf = scratch.tile([P, 4, 512], F32, tag="scratch")
            for k in range(4):
                nc.gpsimd.memset(maskf[:, k, :], 0.0)
                nc.gpsimd.affine_select(
                    out=maskf[:, k, :],
                    in_=maskf[:, k, :],
                    pattern=[[1, 512]],
                    compare_op=mybir.AluOpType.is_ge,
                    fill=NEG,
                    base=-(128 * k),
                    channel_multiplier=-1,
                )
            maskr = singles.tile([P, 4, 512], F32R)
            nc.vector.tensor_copy(maskr, maskf)

            # persistent tensors
            wT = singles.tile([P, CT, 3 * JL], BF16)  # [c, cc, 3j] 24KB/part
            wprojT = singles.tile([P, 4, C], F32R)    # [j, g, c_out] 16KB/part
            k_sb = singles.tile([P, 4, T], F32R)      # [d2, hp, tk]  32KB/part
            v_sb = singles.tile([P, T // P, 8, D + 1], BF16)  # [t, tkt, h, d|1]
            nc.vector.memset(v_sb[:, :, :, D], 1.0)

            # ---- stage 0: weight loads (all direct DMA now) ----
            for cc in range(CT):
                nc.sync.dma_start(
                    out=wT[:, cc, :], in_=wqkvb[cc * P:(cc + 1) * P, :]
                )
            wpst = scratch.tile([P, 4, C], F32, tag="scratch")
            for g in range(4):
                nc.sync.dma_start(
                    out=wpst[:, g, :], in_=wprojb[g * P:(g + 1) * P, :]
                )
            nc.vector.tensor_copy(wprojT, wpst)

            def dma_xT(q):
                """Issue direct DMAs for chunk q's transposed x; returns tile."""
                t0 = q * 512
                xT = xT_pool.tile([P, CT, 512], BF16, tag="xT")
                for cc in range(CT):
                    nc.sync.dma_start(
                        out=xT[:, cc, :],
                        in_=xb[cc * P:(cc + 1) * P, t0:t0 + 512],
                    )
                return xT

            def qkv_ops(q, xT):
                """Closure list emitting QKV(q): v first, then q/k by pair."""
                t0 = q * 512
                q_sb = qsb_pool.tile([P, 4, 512], F32R, tag="qsb")

                def v_group(tt):
                    def emit():
                        pv = ps_mm.tile([P, JL], F32, tag="mm")
                        for cc in range(CT):
                            nc.tensor.matmul(
                                pv,
                                lhsT=xT[:, cc, tt * P:(tt + 1) * P],
                                rhs=wT[:, cc, 2 * JL:3 * JL],
                                start=(cc == 0),
                                stop=(cc == CT - 1),
                            )
                        nc.vector.tensor_copy(
                            out=v_sb[:, q * 4 + tt, :, 0:D],
                            in_=pv.rearrange("p (h d) -> p h d", h=8),
                        )
                    return emit

                def qk_group(jt):
                    def emit():
                        g3, j4 = (0, jt) if jt < 4 else (1, jt - 4)
                        pq = ps_mm.tile([P, 512], F32, tag="mm")
                        for cc in range(CT):
                            nc.tensor.matmul(
                                pq,
                                lhsT=wT[
                                    :, cc,
                                    g3 * JL + j4 * P:g3 * JL + (j4 + 1) * P,
                                ],
                                rhs=xT[:, cc, :],
                                start=(cc == 0),
                                stop=(cc == CT - 1),
                            )
                        if jt < 4:
                            nc.vector.tensor_copy(out=q_sb[:, jt, :], in_=pq)
                        else:
                            nc.vector.tensor_copy(
                                out=k_sb[:, jt - 4, t0:t0 + 512], in_=pq
                            )
                    return emit

                ops = [v_group(tt) for tt in range(4)]
                for hp in range(4):
                    ops.append(qk_group(hp))
                    ops.append(qk_group(hp + 4))
                return q_sb, ops

            def proj_ops(q, yT):
                """Closure list emitting proj(q) (4 mms + cast + DMA each)."""
                t0 = q * 512

                def one(tt, ct):
                    def emit():
                        po = ps_mm.tile([P, 512], F32, tag="mm")
                        for g in range(4):
                            nc.tensor.matmul(
                                po,
                                lhsT=yT[:, g, tt * P:(tt + 1) * P],
                                rhs=wprojT[:, g, ct * 512:(ct + 1) * 512],
                                start=(g == 0),
                                stop=(g == 3),
                            )
                        ob = ob_pool.tile([P, 512], F32, tag="ob")
                        nc.vector.tensor_copy(ob, po)
                        nc.sync.dma_start(
                            out=out[
                                t0 + tt * P:t0 + (tt + 1) * P,
                                ct * 512:(ct + 1) * 512,
                            ],
                            in_=ob,
                        )
                    return emit

                return [one(tt, ct) for tt in range(4) for ct in range(2)]

            pending_div = None

            def emit_div(pend):
                pav0, pav1, yT_t, hp_ = pend
                # l rows (psum row 64) -> sbuf f32r via DVE (keep ACT on exp)
                l2a = small_pool.tile([1, 512], F32R, tag="l2a")
                l2b = small_pool.tile([1, 512], F32R, tag="l2b")
                nc.vector.tensor_copy(l2a, pav0[D:D + 1, :])
                nc.vector.tensor_copy(l2b, pav1[D:D + 1, :])
                pb = ps_mm.tile([P, 512], F32, tag="mm")
                nc.tensor.matmul(pb, lhsT=sel_a, rhs=l2a,
                                 start=True, stop=False)
                nc.tensor.matmul(pb, lhsT=sel_b, rhs=l2b,
                                 start=False, stop=True)
                pbs = small_pool.tile([P, 512], F32, tag="pbs")
                nc.vector.reciprocal_approx_fast(out=pbs, in_=pb)
                nc.vector.tensor_mul(
                    yT_t[0:D, hp_, :], pav0[0:D, :], pbs[0:D, :]
                )
                nc.vector.tensor_mul(
                    yT_t[D:P, hp_, :], pav1[0:D, :], pbs[D:P, :]
                )

            filler = []

            def pull(n):
                for _ in range(min(n, len(filler))):
                    filler.pop(0)()

            # ---- prologue: chunk 0 QKV runs inline ----
            xT_cur = dma_xT(0)
            q_sb_cur, ops0 = qkv_ops(0, xT_cur)
            for op in ops0:
                op()

            for q in range(NCHUNK):
                t0 = q * 512
                # prefetch next chunk's x and queue its QKV as PE filler
                if q + 1 < NCHUNK:
                    xT_nxt = dma_xT(q + 1)
                    q_sb_nxt, opsn = qkv_ops(q + 1, xT_nxt)
                    filler.extend(opsn)

                # ---- attention for tq-chunk q ----
                yT = yT_pool.tile([P, 4, 512], F32R, tag="yT")
                ntk = 4 * q + 4

                for hp in range(NPAIR):
                    pav0 = ps_av.tile([D + 1, 512], F32, tag="av")
                    pav1 = ps_av.tile([D + 1, 512], F32, tag="av")
                    pav = [pav0, pav1]

                    def emit_scores(j, hp=hp, q_sb=q_sb_cur):
                        # diagonal tiles only need columns o:512
                        diag = j >= 4 * q
                        o = j * P - t0 if diag else 0
                        ps = ps_sc.tile([P, 2, 512], F32, tag="sc")
                        for h2 in range(2):
                            nc.tensor.matmul(
                                ps[:, h2, o:512],
                                lhsT=k_sb[
                                    h2 * D:(h2 + 1) * D, hp, j * P:(j + 1) * P
                                ],
                                rhs=q_sb[h2 * D:(h2 + 1) * D, hp, o:512],
                                start=True,
                                stop=not diag,
                            )
                        if diag:  # add causal band mask into psum (128 cols)
                            for h2 in range(2):
                                nc.tensor.matmul(
                                    ps[:, h2, o:o + P],
                                    lhsT=identity_r,
                                    rhs=maskr[:, o // P, o:o + P],
                                    start=False,
                                    stop=True,
                                )
                        return ps, o

                    sc_q = [emit_scores(0)]
                    for j in range(ntk):
                        if j + 1 < ntk:
                            sc_q.append(emit_scores(j + 1))
                        ps, o = sc_q[j]
                        pt = pt_pool.tile([P, 2, 512], BF16, tag="pt")
                        nc.scalar.activation(
                            pt[:, :, o:512], ps[:, :, o:512], Exp, scale=0.125
                        )
                        pull(1)
                        for h2 in range(2):
                            nc.tensor.matmul(
                                pav[h2][:, o:512],
                                lhsT=v_sb[:, j, hp * 2 + h2, :],
                                rhs=pt[:, h2, o:512],
                                start=(j == 0),
                                stop=(j == ntk - 1),
                            )
                    if pending_div is not None:
                        emit_div(pending_div)
                    pending_div = (pav0, pav1, yT, hp)

                if pending_div is not None:
                    emit_div(pending_div)
                    pending_div = None

                # queue proj(q) as filler for the next chunk's attention
                # (chunk 3's proj runs at the end)
                pull(len(filler))
                filler.extend(proj_ops(q, yT))
                if q + 1 < NCHUNK:
                    xT_cur, q_sb_cur = xT_nxt, q_sb_nxt
                else:
                    pull(len(filler))

    nc.compile()
    return nc


_NC = None


def _get_nc():
    global _NC
    if _NC is None:
        _NC = build_nc()
    return _NC


def _shard_inputs(x, w_attn, w_proj):
    in_maps = []
    for c in range(8):
        b, s = c // 2, c % 2
        j0 = s * JL
        wqkv_c = np.concatenate(
            [
                w_attn[j0:j0 + JL],
                w_attn[C + j0:C + j0 + JL],
                w_attn[2 * C + j0:2 * C + j0 + JL],
            ],
            axis=0,
        )
        in_maps.append(
            {
                "xb": np.ascontiguousarray(
                    x[b].astype(ml_dtypes.bfloat16).T
                ),
                "wqkvb": np.ascontiguousarray(
                    wqkv_c.astype(ml_dtypes.bfloat16).T
                ),
                "wprojb": np.ascontiguousarray(
                    w_proj[:, j0:j0 + JL].T
                ).astype(np.float32),
            }
        )
    return in_maps


def run(x, w_attn, w_proj, **run_kwargs):
    """Run on 8 cores; returns (out [B,T,C], BassKernelResults)."""
    nc = _get_nc()
    in_maps = _shard_inputs(np.asarray(x), np.asarray(w_attn), np.asarray(w_proj))
    res = bass_utils.run_bass_kernel_spmd(
        nc, in_maps, core_ids=list(range(8)), **run_kwargs
    )
    out = np.empty((B, T, C), dtype=np.float32)
    for b in range(B):
        out[b] = res.results[2 * b]["out"] + res.results[2 * b + 1]["out"]
    return out, res


def kernel(x, w_attn, w_proj):
    return run(x, w_attn, w_proj)[0]


# revision 4
# speedup vs baseline: 1.4631x; 1.0391x over previous
"""Causal self-attention Bass/Tile kernel for 8 TRN2 NeuronCores.

Sharding: core c handles batch b = c//2 and heads h in [8*(c%2), 8*(c%2)+8).
Each core computes a partial projection output (its 512 channels' worth of the
contraction); the host sums the two partials per batch.

v3 changes vs v2:
  - all matmul operands bf16 (f32r streams the moving operand at 2 cycles/col
    on HW - measured 490ns vs 215ns for N=512 - despite the cost model).
  - causal mask applied by zeroing the exp output's 128-wide diagonal band on
    GPSIMD (idle engine) instead of identity@mask matmuls on the PE.
  - PE warm-up matmuls + reordered DMA (x chunk 0 + wqkv first, wproj last)
    shrink the dead startup window.
  - QKV/proj filler is per-matmul granular, 2 pulled per attention tile, so
    the PE never micro-idles waiting on exp (HAM stays at full clock).

Per-core pipeline (per 512-wide t-chunk):
  xT  [c,t] bf16   <- direct DMA of host-transposed bf16 x
  q,k [j,t] bf16   <- bf16 matmul (lhsT=wT, rhs=xT), psum->sbuf cast
  v   [t,j] bf16   <- bf16 matmul (lhsT=xT, rhs=wT_v), with ones column for l
  S.T [tk,tq] psum <- bf16 matmul, 2 heads row-tiled (auto tile_position)
  P = exp(S/8)     <- single ACT call per tk-tile (both heads), bf16 out;
                      diagonal band zeroed via gpsimd affine_select
  y.T|l  psum      <- bf16 AV matmul accumulation; scores emitted one tile
                      ahead of AV; QKV/proj filler keeps PE busy during exp
  y.T/l            <- l broadcast via K=1 select-matmul, DVE divide
  out [t,c] f32    <- bf16 proj matmul from y.T tiles, DMA to DRAM
"""

import sys

if "/opt/trn_rl_repo" not in sys.path:
    sys.path.insert(0, "/opt/trn_rl_repo")

import ml_dtypes
import numpy as np

import concourse.bass as bass
import concourse.mybir as mybir
import concourse.tile as tile
from concourse import bacc, bass_utils

F32 = mybir.dt.float32
BF16 = mybir.dt.bfloat16

B, T, C = 4, 2048, 1024
H = 16
D = 64
JL = 512          # local channels per q/k/v slice (8 heads * 64)
P = 128
NCHUNK = T // 512
NPAIR = 4


def build_nc():
    nc = bacc.Bacc("TRN2", target_bir_lowering=False, debug=False)
    # host-transposed layouts
    xb = nc.dram_tensor("xb", [C, T], BF16, kind="ExternalInput").ap()
    wqkvb = nc.dram_tensor("wqkvb", [C, 3 * JL], BF16, kind="ExternalInput").ap()
    wprojb = nc.dram_tensor("wprojb", [JL, C], F32, kind="ExternalInput").ap()
    out = nc.dram_tensor("out", [T, C], F32, kind="ExternalOutput").ap()
    wrm = nc.dram_tensor("wrm", [P, 512], F32, kind="ExternalOutput").ap()

    CT = C // P       # 8 c-tiles
    Exp = mybir.ActivationFunctionType.Exp

    with tile.TileContext(nc) as tc:
        with (
            tc.tile_pool(name="singles", bufs=1) as singles,
            tc.tile_pool(name="scratch", bufs=1) as scratch,
            tc.tile_pool(name="xT", bufs=2) as xT_pool,
            tc.tile_pool(name="qsb", bufs=2) as qsb_pool,
            tc.tile_pool(name="pt", bufs=4) as pt_pool,
            tc.tile_pool(name="yT", bufs=2) as yT_pool,
            tc.tile_pool(name="ob", bufs=3) as ob_pool,
            tc.tile_pool(name="small", bufs=2) as small_pool,
            tc.tile_pool(name="ps_sc", bufs=2, space="PSUM") as ps_sc,
            tc.tile_pool(name="ps_av", bufs=2, space="PSUM") as ps_av,
            tc.tile_pool(name="ps_mm", bufs=2, space="PSUM") as ps_mm,
        ):
            # ---- DMAs first so transfers start at t=0 ----
            wT = singles.tile([P, CT, 3 * JL], BF16)  # [c, cc, 3j] 24KB/part
            xT0 = xT_pool.tile([P, CT, 512], BF16, tag="xT")
            for cc in range(CT):
                nc.sync.dma_start(
                    out=xT0[:, cc, :], in_=xb[cc * P:(cc + 1) * P, 0:512]
                )
                nc.sync.dma_start(
                    out=wT[:, cc, :], in_=wqkvb[cc * P:(cc + 1) * P, :]
                )

            # ---- PE warm-up during the DMA window (result discarded) ----
            wsrc = singles.tile([P, 512], BF16)
            nc.vector.memset(wsrc, 0.5)
            pw = ps_mm.tile([P, 512], F32, tag="mm")
            for i in range(16):
                nc.tensor.matmul(
                    pw, lhsT=wsrc[:, 0:P], rhs=wsrc,
                    start=(i == 0), stop=(i == 15),
                )
            wob = ob_pool.tile([P, 512], F32, tag="ob")
            nc.vector.tensor_copy(wob, pw)
            nc.sync.dma_start(out=wrm, in_=wob)

            # head-pair selector for the l broadcast: out rows 0:64 <- l0,
            # rows 64:128 <- l1  (out = sel2.T @ [l0;l1])
            selaf = singles.tile([1, P], F32)
            nc.vector.memset(selaf, 0.0)
            nc.vector.memset(selaf[0:1, 0:D], 1.0)
            selbf = singles.tile([1, P], F32)
            nc.vector.memset(selbf, 0.0)
            nc.vector.memset(selbf[0:1, D:2 * D], 1.0)
            sel_a = singles.tile([1, P], BF16)
            nc.vector.tensor_copy(sel_a, selaf)
            sel_b = singles.tile([1, P], BF16)
            nc.vector.tensor_copy(sel_b, selbf)

            # persistent tensors
            wprojT = singles.tile([P, 4, C], BF16)    # [j, g, c_out]  8KB/part
            k_sb = singles.tile([P, 4, T], BF16)      # [d2, hp, tk]  16KB/part
            v_sb = singles.tile([P, T // P, 8, D + 1], BF16)  # [t, tkt, h, d|1]
            nc.vector.memset(v_sb[:, :, :, D], 1.0)

            def dma_xT(q):
                """Issue direct DMAs for chunk q's transposed x; returns tile."""
                t0 = q * 512
                xT = xT_pool.tile([P, CT, 512], BF16, tag="xT")
                for cc in range(CT):
                    nc.sync.dma_start(
                        out=xT[:, cc, :],
                        in_=xb[cc * P:(cc + 1) * P, t0:t0 + 512],
                    )
                return xT

            def qkv_ops(q, xT):
                """Per-matmul closure list emitting QKV(q): v, then q/k."""
                t0 = q * 512
                q_sb = qsb_pool.tile([P, 4, 512], BF16, tag="qsb")
                ops = []

                def v_mm(tt, cc, pv):
                    def emit():
                        nc.tensor.matmul(
                            pv[0],
                            lhsT=xT[:, cc, tt * P:(tt + 1) * P],
                            rhs=wT[:, cc, 2 * JL:3 * JL],
                            start=(cc == 0),
                            stop=(cc == CT - 1),
                        )
                        if cc == CT - 1:
                            nc.vector.tensor_copy(
                                out=v_sb[:, q * 4 + tt, :, 0:D],
                                in_=pv[0].rearrange("p (h d) -> p h d", h=8),
                            )
                    return emit

                def qk_mm(jt, cc, pq):
                    def emit():
                        g3, j4 = (0, jt) if jt < 4 else (1, jt - 4)
                        nc.tensor.matmul(
                            pq[0],
                            lhsT=wT[
                                :, cc,
                                g3 * JL + j4 * P:g3 * JL + (j4 + 1) * P,
                            ],
                            rhs=xT[:, cc, :],
                            start=(cc == 0),
                            stop=(cc == CT - 1),
                        )
                        if cc == CT - 1:
                            if jt < 4:
                                nc.vector.tensor_copy(
                                    out=q_sb[:, jt, :], in_=pq[0]
                                )
                            else:
                                nc.vector.tensor_copy(
                                    out=k_sb[:, jt - 4, t0:t0 + 512], in_=pq[0]
                                )
                    return emit

                def alloc(pv, shape):
                    def emit():
                        pv[0] = ps_mm.tile(shape, F32, tag="mm", name="pacc")
                    return emit

                for tt in range(4):
                    pv = [None]
                    ops.append(alloc(pv, [P, JL]))
                    for cc in range(CT):
                        ops.append(v_mm(tt, cc, pv))
                for jt in [0, 4, 1, 5, 2, 6, 3, 7]:
                    pq = [None]
                    ops.append(alloc(pq, [P, 512]))
                    for cc in range(CT):
                        ops.append(qk_mm(jt, cc, pq))
                return q_sb, ops

            def proj_ops(q, yT):
                """Per-matmul closure list emitting proj(q)."""
                t0 = q * 512
                ops = []

                def one(tt, ct, g, po):
                    def emit():
                        if g == 0:
                            po[0] = ps_mm.tile([P, 512], F32, tag="mm", name="po")
                        nc.tensor.matmul(
                            po[0],
                            lhsT=yT[:, g, tt * P:(tt + 1) * P],
                            rhs=wprojT[:, g, ct * 512:(ct + 1) * 512],
                            start=(g == 0),
                            stop=(g == 3),
                        )
                        if g == 3:
                            ob = ob_pool.tile([P, 512], F32, tag="ob")
                            nc.vector.tensor_copy(ob, po[0])
                            nc.sync.dma_start(
                                out=out[
                                    t0 + tt * P:t0 + (tt + 1) * P,
                                    ct * 512:(ct + 1) * 512,
                                ],
                                in_=ob,
                            )
                    return emit

                for tt in range(4):
                    for ct in range(2):
                        po = [None]
                        for g in range(4):
                            ops.append(one(tt, ct, g, po))
                return ops

            pending_div = None

            def emit_div(pend):
                pav0, pav1, yT_t, hp_ = pend
                # l rows (psum row 64) -> sbuf bf16 via DVE (keep ACT on exp)
                l2a = small_pool.tile([1, 512], BF16, tag="l2a")
                l2b = small_pool.tile([1, 512], BF16, tag="l2b")
                nc.vector.tensor_copy(l2a, pav0[D:D + 1, :])
                nc.vector.tensor_copy(l2b, pav1[D:D + 1, :])
                pb = ps_mm.tile([P, 512], F32, tag="mm")
                nc.tensor.matmul(pb, lhsT=sel_a, rhs=l2a,
                                 start=True, stop=False)
                nc.tensor.matmul(pb, lhsT=sel_b, rhs=l2b,
                                 start=False, stop=True)
                pbs = small_pool.tile([P, 512], F32, tag="pbs")
                nc.vector.reciprocal_approx_fast(out=pbs, in_=pb)
                nc.vector.tensor_mul(
                    yT_t[0:D, hp_, :], pav0[0:D, :], pbs[0:D, :]
                )
                nc.vector.tensor_mul(
                    yT_t[D:P, hp_, :], pav1[0:D, :], pbs[D:P, :]
                )

            filler = []

            def pull(n):
                for _ in range(min(n, len(filler))):
                    filler.pop(0)()

            # ---- prologue: chunk 0 QKV runs inline ----
            q_sb_cur, ops0 = qkv_ops(0, xT0)
            for op in ops0:
                op()

            # wproj load + bf16 cast (first needed by proj(0) in chunk 1)
            wpst = scratch.tile([P, 4, C], F32, tag="scratch")
            for g in range(4):
                nc.sync.dma_start(
                    out=wpst[:, g, :], in_=wprojb[g * P:(g + 1) * P, :]
                )
            nc.vector.tensor_copy(wprojT, wpst)

            for q in range(NCHUNK):
                t0 = q * 512
                # prefetch next chunk's x and queue its QKV as PE filler
                if q + 1 < NCHUNK:
                    xT_nxt = dma_xT(q + 1)
                    q_sb_nxt, opsn = qkv_ops(q + 1, xT_nxt)
                    filler.extend(opsn)

                # ---- attention for tq-chunk q ----
                yT = yT_pool.tile([P, 4, 512], BF16, tag="yT")
                ntk = 4 * q + 4

                for hp in range(NPAIR):
                    pav0 = ps_av.tile([D + 1, 512], F32, tag="av")
                    pav1 = ps_av.tile([D + 1, 512], F32, tag="av")
                    pav = [pav0, pav1]

                    def emit_scores(j, hp=hp, q_sb=q_sb_cur):
                        # diagonal tiles only need columns o:512
                        diag = j >= 4 * q
                        o = j * P - t0 if diag else 0
                        ps = ps_sc.tile([P, 2, 512], F32, tag="sc")
                        for h2 in range(2):
                            nc.tensor.matmul(
                                ps[:, h2, o:512],
                                lhsT=k_sb[
                                    h2 * D:(h2 + 1) * D, hp, j * P:(j + 1) * P
                                ],
                                rhs=q_sb[h2 * D:(h2 + 1) * D, hp, o:512],
                                start=True,
                                stop=True,
                            )
                        return ps, o

                    sc_q = [emit_scores(0)]
                    for j in range(ntk):
                        if j + 1 < ntk:
                            sc_q.append(emit_scores(j + 1))
                        ps, o = sc_q[j]
                        pt = pt_pool.tile([P, 2, 512], BF16, tag="pt")
                        nc.scalar.activation(
                            pt[:, :, o:512], ps[:, :, o:512], Exp, scale=0.125
                        )
                        diag = j >= 4 * q
                        if diag:
                            # zero the sub-diagonal triangle of the 128-wide
                            # band: keep pt[p, f] only where f >= p
                            for h2 in range(2):
                                nc.gpsimd.affine_select(
                                    out=pt[:, h2, o:o + P],
                                    in_=pt[:, h2, o:o + P],
                                    pattern=[[1, P]],
                                    compare_op=mybir.AluOpType.is_ge,
                                    fill=0.0,
                                    base=0,
                                    channel_multiplier=-1,
                                )
                        pull(2)
                        for h2 in range(2):
                            nc.tensor.matmul(
                                pav[h2][:, o:512],
                                lhsT=v_sb[:, j, hp * 2 + h2, :],
                                rhs=pt[:, h2, o:512],
                                start=(j == 0),
                                stop=(j == ntk - 1),
                            )
                    if pending_div is not None:
                        emit_div(pending_div)
                    pending_div = (pav0, pav1, yT, hp)

                if pending_div is not None:
                    emit_div(pending_div)
                    pending_div = None

                # queue proj(q) as filler for the next chunk's attention
                # (chunk 3's proj runs at the end)
                pull(len(filler))
                filler.extend(proj_ops(q, yT))
                if q + 1 < NCHUNK:
                    xT_cur, q_sb_cur = xT_nxt, q_sb_nxt
                else:
                    pull(len(filler))

    nc.compile()
    return nc


_NC = None


def _get_nc():
    global _NC
    if _NC is None:
        _NC = build_nc()
    return _NC


def _shard_inputs(x, w_attn, w_proj):
    in_maps = []
    for c in range(8):
        b, s = c // 2, c % 2
        j0 = s * JL
        wqkv_c = np.concatenate(
            [
                w_attn[j0:j0 + JL],
                w_attn[C + j0:C + j0 + JL],
                w_attn[2 * C + j0:2 * C + j0 + JL],
            ],
            axis=0,
        )
        in_maps.append(
            {
                "xb": np.ascontiguousarray(
                    x[b].astype(ml_dtypes.bfloat16).T
                ),
                "wqkvb": np.ascontiguousarray(
                    wqkv_c.astype(ml_dtypes.bfloat16).T
                ),
                "wprojb": np.ascontiguousarray(
                    w_proj[:, j0:j0 + JL].T
                ).astype(np.float32),
            }
        )
    return in_maps


def run(x, w_attn, w_proj, **run_kwargs):
    """Run on 8 cores; returns (out [B,T,C], BassKernelResults)."""
    nc = _get_nc()
    in_maps = _shard_inputs(np.asarray(x), np.asarray(w_attn), np.asarray(w_proj))
    res = bass_utils.run_bass_kernel_spmd(
        nc, in_maps, core_ids=list(range(8)), **run_kwargs
    )
    out = np.empty((B, T, C), dtype=np.float32)
    for b in range(B):
        out[b] = res.results[2 * b]["out"] + res.results[2 * b + 1]["out"]
    return out, res


def kernel(x, w_attn, w_proj):
    return run(x, w_attn, w_proj)[0]


# revision 11
# speedup vs baseline: 1.7115x; 1.1698x over previous
"""Causal self-attention Bass/Tile kernel for 8 TRN2 NeuronCores.

Sharding: core c handles batch b = c//2 and heads h in [8*(c%2), 8*(c%2)+8).
Each core computes a partial projection output (its 512 channels' worth of the
contraction); the host sums the two partials per batch.

v3 changes vs v2:
  - all matmul operands bf16 (f32r streams the moving operand at 2 cycles/col
    on HW - measured 490ns vs 215ns for N=512 - despite the cost model).
  - causal band mask via identity@mask matmuls over only the 128-wide
    diagonal band, emitted after the score matmuls (accumulate into psum).
    (A gpsimd post-exp zeroing variant delayed AV readiness and made the
    scheduler split the row-packed score pairs - net loss.)
  - PE warm-up matmuls + reordered DMA (x chunk 0 + wqkv first, wproj last)
    shrink the dead startup window.
  - QKV/proj filler is per-matmul granular, 2 pulled per attention tile, so
    the PE never micro-idles waiting on exp (HAM stays at full clock).

Per-core pipeline (per 512-wide t-chunk):
  xT  [c,t] bf16   <- direct DMA of host-transposed bf16 x
  q,k [j,t] bf16   <- bf16 matmul (lhsT=wT, rhs=xT), psum->sbuf cast
  v   [t,j] bf16   <- bf16 matmul (lhsT=xT, rhs=wT_v), with ones column for l
  S.T [tk,tq] psum <- bf16 matmul, 2 heads row-tiled (auto tile_position)
  P = exp(S/8)     <- single ACT call per tk-tile (both heads), bf16 out;
                      diagonal band zeroed via gpsimd affine_select
  y.T|l  psum      <- bf16 AV matmul accumulation; scores emitted one tile
                      ahead of AV; QKV/proj filler keeps PE busy during exp
  y.T/l            <- l broadcast via K=1 select-matmul, DVE divide
  out [t,c] f32    <- bf16 proj matmul from y.T tiles, DMA to DRAM
"""

import sys

if "/opt/trn_rl_repo" not in sys.path:
    sys.path.insert(0, "/opt/trn_rl_repo")

import ml_dtypes
import numpy as np

import concourse.bass as bass
import concourse.mybir as mybir
import concourse.tile as tile
from concourse import bacc, bass_utils
from concourse.masks import make_identity

F32 = mybir.dt.float32
BF16 = mybir.dt.bfloat16

B, T, C = 4, 2048, 1024
H = 16
D = 64
JL = 512          # local channels per q/k/v slice (8 heads * 64)
P = 128
NCHUNK = T // 512
NPAIR = 4


def build_nc():
    nc = bacc.Bacc("TRN2", target_bir_lowering=False, debug=False)
    # host-transposed layouts
    xb = nc.dram_tensor("xb", [C, T], BF16, kind="ExternalInput").ap()
    wqkvb = nc.dram_tensor("wqkvb", [C, 3 * JL], BF16, kind="ExternalInput").ap()
    wprojb = nc.dram_tensor("wprojb", [JL, C], F32, kind="ExternalInput").ap()
    out = nc.dram_tensor("out", [T, C], F32, kind="ExternalOutput").ap()
    wrm = nc.dram_tensor("wrm", [P, 512], F32, kind="ExternalOutput").ap()

    CT = C // P       # 8 c-tiles
    Exp = mybir.ActivationFunctionType.Exp

    with tile.TileContext(nc) as tc:
        with (
            tc.tile_pool(name="singles", bufs=1) as singles,
            tc.tile_pool(name="scratch", bufs=1) as scratch,
            tc.tile_pool(name="xT", bufs=2) as xT_pool,
            tc.tile_pool(name="qsb", bufs=2) as qsb_pool,
            tc.tile_pool(name="pt", bufs=4) as pt_pool,
            tc.tile_pool(name="yT", bufs=2) as yT_pool,
            tc.tile_pool(name="ob", bufs=3) as ob_pool,
            tc.tile_pool(name="small", bufs=2) as small_pool,
            tc.tile_pool(name="ps_sc", bufs=2, space="PSUM") as ps_sc,
            tc.tile_pool(name="ps_av", bufs=2, space="PSUM") as ps_av,
            tc.tile_pool(name="ps_mm", bufs=2, space="PSUM") as ps_mm,
        ):
            # ---- DMAs first so transfers start at t=0 ----
            # v-slice of wqkv first so the first v matmul group can begin
            # as early as possible, then x chunk 0 + the q/k slices.
            wT = singles.tile([P, CT, 3 * JL], BF16)  # [c, cc, 3j] 24KB/part
            xT0 = xT_pool.tile([P, CT, 512], BF16, tag="xT")
            for cc in range(CT):
                nc.sync.dma_start(
                    out=xT0[:, cc, :], in_=xb[cc * P:(cc + 1) * P, 0:512]
                )
                nc.sync.dma_start(
                    out=wT[:, cc, 2 * JL:3 * JL],
                    in_=wqkvb[cc * P:(cc + 1) * P, 2 * JL:3 * JL],
                )
            for cc in range(CT):
                nc.sync.dma_start(
                    out=wT[:, cc, 0:2 * JL],
                    in_=wqkvb[cc * P:(cc + 1) * P, 0:2 * JL],
                )

            # ---- PE warm-up during the DMA window (result discarded) ----
            wsrc = singles.tile([P, 512], BF16)
            nc.vector.memset(wsrc, 0.5)
            pw = ps_mm.tile([P, 512], F32, tag="mm")
            for i in range(12):
                nc.tensor.matmul(
                    pw, lhsT=wsrc[:, 0:P], rhs=wsrc,
                    start=(i == 0), stop=(i == 11),
                )
            wob = ob_pool.tile([P, 512], F32, tag="ob")
            nc.vector.tensor_copy(wob, pw)
            nc.sync.dma_start(out=wrm, in_=wob)

            # head-pair selector for the l broadcast: out rows 0:64 <- l0,
            # rows 64:128 <- l1  (out = sel2.T @ [l0;l1])
            selaf = singles.tile([1, P], F32)
            nc.vector.memset(selaf, 0.0)
            nc.vector.memset(selaf[0:1, 0:D], 1.0)
            selbf = singles.tile([1, P], F32)
            nc.vector.memset(selbf, 0.0)
            nc.vector.memset(selbf[0:1, D:2 * D], 1.0)
            sel_a = singles.tile([1, P], BF16)
            nc.vector.tensor_copy(sel_a, selaf)
            sel_b = singles.tile([1, P], BF16)
            nc.vector.tensor_copy(sel_b, selbf)

            # identity (bf16) + additive causal band masks, one per diagonal
            # offset o=128*k: mask[k][p, f] = 0 if f >= p + 128k else -1e30
            identf = singles.tile([P, P], F32)
            make_identity(nc, identf)
            identity = singles.tile([P, P], BF16)
            nc.vector.tensor_copy(identity, identf)
            maskf = scratch.tile([P, P], F32, tag="maskf")
            nc.gpsimd.memset(maskf, 0.0)
            nc.gpsimd.affine_select(
                out=maskf,
                in_=maskf,
                pattern=[[1, P]],
                compare_op=mybir.AluOpType.is_ge,
                fill=-1e30,
                base=0,
                channel_multiplier=-1,
            )
            maskr = singles.tile([P, P], BF16)
            nc.vector.tensor_copy(maskr, maskf)

            # persistent tensors
            wprojT = singles.tile([P, 4, C], BF16)    # [j, g, c_out]  8KB/part
            k_sb = singles.tile([P, 4, T], BF16)      # [d2, hp, tk]  16KB/part
            v_sb = singles.tile([P, T // P, 8, D + 1], BF16)  # [t, tkt, h, d|1]
            nc.vector.memset(v_sb[:, :, :, D], 1.0)

            def dma_xT(q):
                """Issue direct DMAs for chunk q's transposed x; returns tile."""
                t0 = q * 512
                xT = xT_pool.tile([P, CT, 512], BF16, tag="xT")
                for cc in range(CT):
                    nc.sync.dma_start(
                        out=xT[:, cc, :],
                        in_=xb[cc * P:(cc + 1) * P, t0:t0 + 512],
                    )
                return xT

            def qkv_ops(q, xT):
                """Per-matmul closure list emitting QKV(q): v, then q/k."""
                t0 = q * 512
                q_sb = qsb_pool.tile([P, 4, 512], BF16, tag="qsb")
                ops = []

                def v_mm(tt, cc, pv):
                    def emit():
                        nc.tensor.matmul(
                            pv[0],
                            lhsT=xT[:, cc, tt * P:(tt + 1) * P],
                            rhs=wT[:, cc, 2 * JL:3 * JL],
                            start=(cc == 0),
                            stop=(cc == CT - 1),
                        )
                        if cc == CT - 1:
                            nc.vector.tensor_copy(
                                out=v_sb[:, q * 4 + tt, :, 0:D],
                                in_=pv[0].rearrange("p (h d) -> p h d", h=8),
                            )
                    return emit

                def qk_mm(jt, cc, pq):
                    def emit():
                        g3, j4 = (0, jt) if jt < 4 else (1, jt - 4)
                        nc.tensor.matmul(
                            pq[0],
                            lhsT=wT[
                                :, cc,
                                g3 * JL + j4 * P:g3 * JL + (j4 + 1) * P,
                            ],
                            rhs=xT[:, cc, :],
                            start=(cc == 0),
                            stop=(cc == CT - 1),
                        )
                        if cc == CT - 1:
                            if jt < 4:
                                nc.vector.tensor_copy(
                                    out=q_sb[:, jt, :], in_=pq[0]
                                )
                            else:
                                nc.vector.tensor_copy(
                                    out=k_sb[:, jt - 4, t0:t0 + 512], in_=pq[0]
                                )
                    return emit

                def alloc(pv, shape):
                    def emit():
                        pv[0] = ps_mm.tile(shape, F32, tag="mm", name="pacc")
                    return emit

                for tt in range(4):
                    pv = [None]
                    ops.append(alloc(pv, [P, JL]))
                    for cc in range(CT):
                        ops.append(v_mm(tt, cc, pv))
                for jt in [0, 4, 1, 5, 2, 6, 3, 7]:
                    pq = [None]
                    ops.append(alloc(pq, [P, 512]))
                    for cc in range(CT):
                        ops.append(qk_mm(jt, cc, pq))
                return q_sb, ops

            def proj_ops(q, yT):
                """Per-matmul closure list emitting proj(q)."""
                t0 = q * 512
                ops = []

                def one(tt, ct, g, po):
                    def emit():
                        if g == 0:
                            po[0] = ps_mm.tile([P, 512], F32, tag="mm", name="po")
                        nc.tensor.matmul(
                            po[0],
                            lhsT=yT[:, g, tt * P:(tt + 1) * P],
                            rhs=wprojT[:, g, ct * 512:(ct + 1) * 512],
                            start=(g == 0),
                            stop=(g == 3),
                        )
                        if g == 3:
                            ob = ob_pool.tile([P, 512], F32, tag="ob")
                            nc.vector.tensor_copy(ob, po[0])
                            nc.sync.dma_start(
                                out=out[
                                    t0 + tt * P:t0 + (tt + 1) * P,
                                    ct * 512:(ct + 1) * 512,
                                ],
                                in_=ob,
                            )
                    return emit

                for tt in range(4):
                    for ct in range(2):
                        po = [None]
                        for g in range(4):
                            ops.append(one(tt, ct, g, po))
                return ops

            pending_div = None

            def emit_div(pend):
                pav0, pav1, yT_t, hp_ = pend
                # l rows (psum row 64) -> sbuf bf16 via DVE (keep ACT on exp)
                l2a = small_pool.tile([1, 512], BF16, tag="l2a")
                l2b = small_pool.tile([1, 512], BF16, tag="l2b")
                nc.vector.tensor_copy(l2a, pav0[D:D + 1, :])
                nc.vector.tensor_copy(l2b, pav1[D:D + 1, :])
                pb = ps_mm.tile([P, 512], F32, tag="mm")
                nc.tensor.matmul(pb, lhsT=sel_a, rhs=l2a,
                                 start=True, stop=False)
                nc.tensor.matmul(pb, lhsT=sel_b, rhs=l2b,
                                 start=False, stop=True)
                pbs = small_pool.tile([P, 512], F32, tag="pbs")
                nc.vector.reciprocal_approx_fast(out=pbs, in_=pb)
                nc.vector.tensor_mul(
                    yT_t[0:D, hp_, :], pav0[0:D, :], pbs[0:D, :]
                )
                nc.vector.tensor_mul(
                    yT_t[D:P, hp_, :], pav1[0:D, :], pbs[D:P, :]
                )

            filler = []

            def pull(n):
                for _ in range(min(n, len(filler))):
                    filler.pop(0)()

            # ---- prologue: chunk 0 QKV runs inline ----
            q_sb_cur, ops0 = qkv_ops(0, xT0)
            for op in ops0:
                op()

            # wproj load + bf16 cast (first needed by proj(0) in chunk 1)
            wpst = scratch.tile([P, 4, C], F32, tag="scratch")
            for g in range(4):
                nc.sync.dma_start(
                    out=wpst[:, g, :], in_=wprojb[g * P:(g + 1) * P, :]
                )
            nc.vector.tensor_copy(wprojT, wpst)

            for q in range(NCHUNK):
                t0 = q * 512
                # prefetch next chunk's x and queue its QKV as PE filler
                if q + 1 < NCHUNK:
                    xT_nxt = dma_xT(q + 1)
                    q_sb_nxt, opsn = qkv_ops(q + 1, xT_nxt)
                    filler.extend(opsn)

                # ---- attention for tq-chunk q ----
                yT = yT_pool.tile([P, 4, 512], BF16, tag="yT")
                ntk = 4 * q + 4

                for hp in range(NPAIR):
                    pav0 = ps_av.tile([D + 1, 512], F32, tag="av")
                    pav1 = ps_av.tile([D + 1, 512], F32, tag="av")
                    pav = [pav0, pav1]

                    def emit_scores(j, hp=hp, q_sb=q_sb_cur):
                        # diagonal tiles only need columns o:512
                        diag = j >= 4 * q
                        o = j * P - t0 if diag else 0
                        ps = ps_sc.tile([P, 2, 512], F32, tag="sc")
                        for h2 in range(2):
                            nc.tensor.matmul(
                                ps[:, h2, o:512],
                                lhsT=k_sb[
                                    h2 * D:(h2 + 1) * D, hp, j * P:(j + 1) * P
                                ],
                                rhs=q_sb[h2 * D:(h2 + 1) * D, hp, o:512],
                                start=True,
                                stop=not diag,
                            )
                        if diag:  # add causal band mask into psum (128 cols)
                            for h2 in range(2):
                                nc.tensor.matmul(
                                    ps[:, h2, o:o + P],
                                    lhsT=identity,
                                    rhs=maskr,
                                    start=False,
                                    stop=True,
                                )
                        return ps, o

                    sc_q = [emit_scores(0)]
                    for j in range(ntk):
                        if j + 1 < ntk:
                            sc_q.append(emit_scores(j + 1))
                        ps, o = sc_q[j]
                        pt = pt_pool.tile([P, 2, 512], BF16, tag="pt")
                        nc.scalar.activation(
                            pt[:, :, o:512], ps[:, :, o:512], Exp, scale=0.125
                        )
                        pull(2)
                        for h2 in range(2):
                            nc.tensor.matmul(
                                pav[h2][:, o:512],
                                lhsT=v_sb[:, j, hp * 2 + h2, :],
                                rhs=pt[:, h2, o:512],
                                start=(j == 0),
                                stop=(j == ntk - 1),
                            )
                    if pending_div is not None:
                        emit_div(pending_div)
                    pending_div = (pav0, pav1, yT, hp)

                if pending_div is not None:
                    emit_div(pending_div)
                    pending_div = None

                # queue proj(q) as filler for the next chunk's attention
                # (chunk 3's proj runs at the end)
                pull(len(filler))
                filler.extend(proj_ops(q, yT))
                if q + 1 < NCHUNK:
                    xT_cur, q_sb_cur = xT_nxt, q_sb_nxt
                else:
                    pull(len(filler))

    nc.compile()
    return nc


_NC = None


def _get_nc():
    global _NC
    if _NC is None:
        _NC = build_nc()
    return _NC


def _shard_inputs(x, w_attn, w_proj):
    in_maps = []
    for c in range(8):
        b, s = c // 2, c % 2
        j0 = s * JL
        wqkv_c = np.concatenate(
            [
                w_attn[j0:j0 + JL],
                w_attn[C + j0:C + j0 + JL],
                w_attn[2 * C + j0:2 * C + j0 + JL],
            ],
            axis=0,
        )
        in_maps.append(
            {
                "xb": np.ascontiguousarray(
                    x[b].astype(ml_dtypes.bfloat16).T
                ),
                "wqkvb": np.ascontiguousarray(
                    wqkv_c.astype(ml_dtypes.bfloat16).T
                ),
                "wprojb": np.ascontiguousarray(
                    w_proj[:, j0:j0 + JL].T
                ).astype(np.float32),
            }
        )
    return in_maps


def run(x, w_attn, w_proj, **run_kwargs):
    """Run on 8 cores; returns (out [B,T,C], BassKernelResults)."""
    nc = _get_nc()
    in_maps = _shard_inputs(np.asarray(x), np.asarray(w_attn), np.asarray(w_proj))
    res = bass_utils.run_bass_kernel_spmd(
        nc, in_maps, core_ids=list(range(8)), **run_kwargs
    )
    out = np.empty((B, T, C), dtype=np.float32)
    for b in range(B):
        out[b] = res.results[2 * b]["out"] + res.results[2 * b + 1]["out"]
    return out, res


def kernel(x, w_attn, w_proj):
    return run(x, w_attn, w_proj)[0]
